# revision 64
# baseline (speedup 1.0000x reference)
"""MDLSTM cell (2-direction) Bass/Tile kernel for Trainium2, 8-core SPMD.

Math (per direction d, with shared input projections):
    i = sigmoid(w_ii @ x + w_hi @ h_d + b_i)
    f = sigmoid(w_if @ x + w_hf @ h_d + b_f)
    g = tanh   (w_ig @ x + w_hg @ h_d + b_g)
    o = sigmoid(w_io @ x + w_ho @ h_d + b_o)
    c_d = f * c_prev_d + i * g
    h_d = o * tanh(c_d)
ct = ws0 * c_0 + ws1 * c_1 ;  ht = ws0 * h_0 + ws1 * h_1

Sharding: all activations/states split along N (=8192) across 8 cores;
weights replicated. No cross-core communication.

Per-core kernel: per output row tile (M=128) the 4 shared input
projections are computed once into PSUM (start=True groups) and copied to
SBUF; each of the 8 gate/direction accumulations then starts by injecting
that x-projection into its PSUM bank via a VectorE copy and accumulates
the hidden projection on top (start=False matmuls — PE-write accumulate
onto engine-written PSUM, valid because every bank's first group in
program order is a start=True group that defines has_written). ScalarE
applies sigmoid/tanh + per-partition bias straight out of PSUM; VectorE
does the elementwise cell update and direction combine.

Performance design (~246us HW, vs 342.9us fp32r baseline = 1.39x):
- fp16 matmul operands (1 col/cycle like bf16/fp32r) with per-gate fp8e4
  DoubleRow hidden projections (2 k-tiles per pass, 2 MACs/cell/cycle;
  HW-measured ~230ns vs 2x216ns). fp8 k-tiles allocated by per-gate error
  sensitivity (i=8, f=8, g=0, o=4 of 8): the g-gate is ~7x costlier per
  fp8 k-tile than i because tanh' reaches 1.0 vs sigmoid's 0.25. Unscaled
  e4m3 (PSUM group accumulation forbids per-matmul rescale; h fits
  normals, the small xavier weights land in subnormals which act as a
  uniform quantizer). Deterministic end-to-end rel_fro = 1.5352e-2 vs the
  2e-2 gate; HW matches the numpy quantization sim to 5 digits.
- Engine balance: once the matmul stream shrank, DVE became co-bottleneck
  at ~95%, so the g-gate PSUM injects run on ScalarE, the direction
  weight ws_d is folded into the ig/fc/hnew scalar_tensor_tensor ops
  (tanh un-scales exactly via the activation input scale = 1/ws_d), and
  the per-direction c/h come out pre-scaled so the combine is one add per
  output. DVE/ACT land at ~75-78% of the PE stream.
- x-projections software-pipelined one m-tile ahead as PE filler work;
  DMAs issued strictly in first-use order, one issue per tensor chunk
  (each dma_start costs ~0.8us serial Sync-sequencer time, so startup is
  issue-count-bound; weights are one contiguous DMA per m-tile); c_prev
  chunks prefetched a full m-tile ahead.
- ~2.5us of throwaway fp32 matmuls on the bias tile warm the PE HAM
  clock gate (cold default is 1.2GHz, ~3.4us busy to reach 2.4GHz)
  during the initial DMA wait, so the real stream starts warm.
- Gate order i,g,f,o per direction so ig, f*c_prev and tanh(c) all
  complete under the o-gate's matmul group: post-last-matmul tail is
  just ACT(o) -> o*tanh(c) -> combine -> DMA.
"""

import numpy as np

import concourse.bass as bass  # noqa: F401  (bass types via bacc/tile)
import concourse.mybir as mybir
import concourse.tile as tile
from concourse import bacc
from concourse.bass_utils import run_bass_kernel_spmd

N_CORES = 8
IN_C = 512
OUT_C = 1024
N = 8192
NS = N // N_CORES  # columns per core
NCH = 512  # psum free-dim chunk (one bank)
N_CHUNKS = NS // NCH
KX = IN_C // 128  # k-tiles of the input projection
KH = OUT_C // 128  # k-tiles of the hidden projection
M_TILES = OUT_C // 128
# Per-gate fp8 allocation: NH8[g] leading hidden k-tiles of gate g go
# through fp8e4 DoubleRow matmuls (2 k-tiles per pass, 2 MACs/cell/cycle);
# the rest stay fp16. Unscaled e4m3 (PSUM accumulation forbids per-matmul
# rescale). Allocation follows per-gate error sensitivity (exact numpy
# sim, which matches HW to 5 digits): err^2 per fp8 k-tile is i:7.7e-6
# f:14.2e-6 o:21.1e-6 g:53.3e-6 (tanh' reaches 1.0 vs sigmoid's 0.25, so
# the g gate is 7x costlier than i). i=8,f=8,g=0,o=4 sims at rel_fro
# 1.535e-2 vs the 2e-2 gate.
NH8 = (8, 8, 0, 4)  # fp8 k-tiles per gate (i, f, g, o); even values
KHF_G = tuple(KH - v for v in NH8)  # fp16 k-tiles per gate
PAIRS = tuple(v // 2 for v in NH8)  # DoubleRow passes per gate
P8_TOT = sum(PAIRS)
F16_TOT = sum(KHF_G)
OFF8 = tuple(sum(PAIRS[:g]) for g in range(4))  # pair offset per gate
OFFF = tuple(sum(KHF_G[:g]) for g in range(4))  # fp16 k-tile offset

F32 = mybir.dt.float32
MM_MODE = "fp16"  # one of: "fp32r", "bf16", "fp16"
import ml_dtypes as _mld
MM_DT = {"fp32r": mybir.dt.float32r, "bf16": mybir.dt.bfloat16,
         "fp16": mybir.dt.float16}[MM_MODE]
MM_NP = {"fp32r": np.float32, "bf16": _mld.bfloat16,
         "fp16": np.float16}[MM_MODE]

SIG = mybir.ActivationFunctionType.Sigmoid
TANH = mybir.ActivationFunctionType.Tanh
MULT = mybir.AluOpType.mult
ADD = mybir.AluOpType.add
COPY = mybir.ActivationFunctionType.Copy


def _build(ws0: float, ws1: float):
    nc = bacc.Bacc(
        "TRN2", target_bir_lowering=False, debug=False, num_devices=N_CORES
    )

    F8 = mybir.dt.float8e4
    # Activations are stored per 512-column chunk, fully contiguous, so each
    # chunk is ONE dma_start: the Sync sequencer spends ~0.8us of serial time
    # per issued DMA and the startup critical path is issue-count-bound.
    xd_ = [
        nc.dram_tensor(f"x{n}", [128, KX, NCH], MM_DT, kind="ExternalInput")
        for n in range(N_CHUNKS)
    ]
    hd_ = [
        [
            nc.dram_tensor(f"h{d}_{n}", [128, KH, NCH], MM_DT, kind="ExternalInput")
            for n in range(N_CHUNKS)
        ]
        for d in (0, 1)
    ]
    h8d_ = [
        [
            nc.dram_tensor(f"h8_{d}_{n}", [128, KH, NCH], F8, kind="ExternalInput")
            for n in range(N_CHUNKS)
        ]
        for d in (0, 1)
    ]
    cd_ = [
        nc.dram_tensor(f"c{d}", [OUT_C, NS], F32, kind="ExternalInput")
        for d in (0, 1)
    ]
    # weights: [m_tile, partition(k%128), gate, k_tile, m_in_tile] — one
    # contiguous DMA per m-tile covering all 4 gates.
    wxd = nc.dram_tensor("wx", [M_TILES, 128, 4, KX, 128], MM_DT, kind="ExternalInput")
    whd = nc.dram_tensor(
        "wh", [M_TILES, 128, F16_TOT, 128], MM_DT, kind="ExternalInput"
    )
    wh8d = nc.dram_tensor(
        "wh8", [M_TILES, 128, P8_TOT, 2, 128], F8, kind="ExternalInput"
    )
    biasd = nc.dram_tensor("bias", [128, 4 * M_TILES], F32, kind="ExternalInput")
    ctd = nc.dram_tensor("ct", [OUT_C, NS], F32, kind="ExternalOutput")
    htd = nc.dram_tensor("ht", [OUT_C, NS], F32, kind="ExternalOutput")

    with tile.TileContext(nc) as tc:
        with (
            tc.tile_pool(name="resident", bufs=1) as res_pool,
            tc.tile_pool(name="wx", bufs=3) as wx_pool,
            tc.tile_pool(name="wh", bufs=4) as wh_pool,
            tc.tile_pool(name="wh8", bufs=4) as wh8_pool,
            tc.tile_pool(name="psum", bufs=8, space="PSUM") as ps_pool,
            tc.tile_pool(name="xproj", bufs=20) as xp_pool,
            tc.tile_pool(name="gates", bufs=6) as g_pool,
            tc.tile_pool(name="cprev", bufs=6) as cp_pool,
            tc.tile_pool(name="tmp", bufs=2) as t_pool,
            tc.tile_pool(name="dirres", bufs=4) as dr_pool,
            tc.tile_pool(name="out", bufs=2) as o_pool,
        ):
            wx_tiles: dict = {}
            wh_tiles: dict = {}
            wh8_tiles: dict = {}

            def load_wx(mt):
                wx_tiles[mt] = wx_pool.tile(
                    [128, 4, KX, 128], MM_DT, tag="wx", name=f"wx_{mt}"
                )
                nc.sync.dma_start(wx_tiles[mt][:], wxd[mt])

            def load_wh8(mt):
                wh8_tiles[mt] = wh8_pool.tile(
                    [128, P8_TOT, 2, 128], F8, tag="wh8", name=f"wh8_{mt}"
                )
                nc.sync.dma_start(wh8_tiles[mt][:], wh8d[mt])

            def load_wh(mt):
                wh_tiles[mt] = wh_pool.tile(
                    [128, F16_TOT, 128], MM_DT, tag="wh", name=f"wh_{mt}"
                )
                nc.sync.dma_start(wh_tiles[mt][:], whd[mt])

            def load_w(mt):
                load_wx(mt)
                load_wh8(mt)
                load_wh(mt)

            x_sb = [
                res_pool.tile([128, KX, NCH], MM_DT, tag=f"x{n}", name=f"x_sb{n}")
                for n in range(N_CHUNKS)
            ]
            h_sb = [
                [
                    res_pool.tile(
                        [128, KH, NCH], MM_DT, tag=f"h{d}_{n}", name=f"h_sb{d}_{n}"
                    )
                    for n in range(N_CHUNKS)
                ]
                for d in (0, 1)
            ]
            h8_sb = [
                [
                    res_pool.tile(
                        [128, KH, NCH], F8, tag=f"h8_{d}_{n}", name=f"h8_sb{d}_{n}"
                    )
                    for n in range(N_CHUNKS)
                ]
                for d in (0, 1)
            ]
            bias_sb = res_pool.tile([128, 4 * M_TILES], F32, tag="bias")

            # Startup: DMAs strictly in first-use order, one issue per
            # tensor-chunk. The PE's early work queue is px(0) then px(1)
            # (x-projections, needing only wx+x); the first hidden group
            # additionally needs wh8[0]+h8[d0]n0 (DR matmul comes first in
            # the group) then wh[0]+h[d0]n0. Meanwhile ~5us of throwaway
            # fp32 matmuls on the bias tile warm the PE HAM clock gate (idle
            # default is 1.2GHz; it takes ~3.4us of busy PE to unthrottle to
            # 2.4GHz) so the real stream starts warm.
            nc.sync.dma_start(bias_sb[:], biasd[:])
            load_wx(0)
            nc.sync.dma_start(x_sb[0][:], xd_[0][:])

            warm_ps = ps_pool.tile([128, NCH], F32, tag="ps", name="warm_ps")
            N_WARM = 24
            for i in range(N_WARM):
                nc.tensor.matmul(
                    warm_ps[:32, :32],
                    bias_sb[:, :32],
                    bias_sb[:, :32],
                    start=(i == 0),
                    stop=(i == N_WARM - 1),
                )

            load_wx(1)
            nc.sync.dma_start(x_sb[1][:], xd_[1][:])
            load_wh8(0)
            load_wh(0)
            nc.sync.dma_start(h8_sb[0][0][:], h8d_[0][0][:])
            nc.sync.dma_start(h_sb[0][0][:], hd_[0][0][:])
            nc.sync.dma_start(h8_sb[1][0][:], h8d_[1][0][:])
            nc.sync.dma_start(h_sb[1][0][:], hd_[1][0][:])
            nc.sync.dma_start(h8_sb[0][1][:], h8d_[0][1][:])
            nc.sync.dma_start(h_sb[0][1][:], hd_[0][1][:])
            nc.sync.dma_start(h8_sb[1][1][:], h8d_[1][1][:])
            nc.sync.dma_start(h_sb[1][1][:], hd_[1][1][:])
            load_wh8(1)
            load_wh(1)

            def px_phase(mt, ci, wxm):
                xp = []
                for g in range(4):
                    px = ps_pool.tile(
                        [128, NCH], F32, tag="ps", name=f"px_{mt}_{ci}_{g}"
                    )
                    for kt in range(KX):
                        nc.tensor.matmul(
                            px[:],
                            wxm[:, g, kt, :],
                            x_sb[ci][:, kt, :],
                            start=(kt == 0),
                            stop=(kt == KX - 1),
                        )
                    xpt = xp_pool.tile(
                        [128, NCH], F32, tag="xp", name=f"xp_{mt}_{ci}_{g}"
                    )
                    nc.scalar.activation(xpt[:], px[:], COPY)
                    xp.append(xpt)
                return xp

            def dir_phase(mt, ci, d, xp, whm, wh8m, msl, cp):
                wsd = ws0 if d == 0 else ws1
                # Gate order i, g, f, o: i*g and f*c_prev + tanh(c) complete
                # while the o-gate's matmul group still runs, so the
                # post-last-matmul chain is just ACT(o) -> o*tanh(c) ->
                # combine (shortest tail on the final chunk).
                # The direction weight ws_d is folded into ig/fc via
                # scalar_tensor_tensor, so cnew/hnew come out pre-scaled and
                # the combine is a single add per output; tanh un-scales via
                # the activation's input scale (exact: tanh(ws*c * 1/ws)).
                gt = {}
                for g in (0, 2, 1, 3):
                    ps = ps_pool.tile(
                        [128, NCH], F32, tag="ps", name=f"ps_{mt}_{ci}_{d}_{g}"
                    )
                    # inject the shared x-projection, then accumulate the
                    # hidden projection on top of it: this gate's first
                    # NH8[g] k-tiles as fp8 DoubleRow passes (2 k-tiles
                    # each), the remaining k-tiles in fp16. The g-gate's
                    # inject runs on ScalarE to balance DVE/ACT load (DVE
                    # was the co-bottleneck at ~95% once the matmul stream
                    # shrank).
                    if g == 2:
                        nc.scalar.activation(ps[:], xp[g][:], COPY)
                    else:
                        nc.vector.tensor_copy(ps[:], xp[g][:])
                    for j in range(PAIRS[g]):
                        nc.tensor.matmul(
                            ps[:],
                            wh8m[:, OFF8[g] + j],
                            h8_sb[d][ci][:, 2 * j : 2 * j + 2, :],
                            start=False,
                            stop=(KHF_G[g] == 0 and j == PAIRS[g] - 1),
                            skip_group_check=True,
                            perf_mode=mybir.MatmulPerfMode.DoubleRow,
                        )
                    for kh in range(KHF_G[g]):
                        nc.tensor.matmul(
                            ps[:],
                            whm[:, OFFF[g] + kh, :],
                            h_sb[d][ci][:, NH8[g] + kh, :],
                            start=False,
                            stop=(kh == KHF_G[g] - 1),
                            skip_group_check=True,
                        )
                    gact = g_pool.tile(
                        [128, NCH], F32, tag="gate", name=f"gate_{mt}_{ci}_{d}_{g}"
                    )
                    nc.scalar.activation(
                        gact[:],
                        ps[:],
                        TANH if g == 2 else SIG,
                        bias=bias_sb[:, g * M_TILES + mt : g * M_TILES + mt + 1],
                    )
                    gt[g] = gact
                    if g == 2:
                        # ig_scaled = (i * ws_d) * g
                        ig = t_pool.tile([128, NCH], F32, tag="ig")
                        nc.vector.scalar_tensor_tensor(
                            ig[:], gt[0][:], wsd, gt[2][:], MULT, MULT
                        )
                    elif g == 1:
                        # fc_scaled = (f * ws_d) * c_prev
                        fc = t_pool.tile([128, NCH], F32, tag="fc")
                        nc.vector.scalar_tensor_tensor(
                            fc[:], gt[1][:], wsd, cp[:], MULT, MULT
                        )
                        cnew = dr_pool.tile([128, NCH], F32, tag="cnew")
                        nc.vector.tensor_add(cnew[:], ig[:], fc[:])
                        tch = t_pool.tile([128, NCH], F32, tag="tch")
                        nc.scalar.activation(
                            tch[:], cnew[:], TANH, scale=1.0 / wsd
                        )

                # hnew_scaled = (o * ws_d) * tanh(c)
                hnew = dr_pool.tile([128, NCH], F32, tag="hnew")
                nc.vector.scalar_tensor_tensor(
                    hnew[:], gt[3][:], wsd, tch[:], MULT, MULT
                )
                return cnew, hnew

            def combine(ci, msl, cdir, hdir):
                nsl = slice(ci * NCH, (ci + 1) * NCH)
                ctt = o_pool.tile([128, NCH], F32, tag="ctt")
                nc.vector.tensor_add(ctt[:], cdir[0][:], cdir[1][:])
                nc.sync.dma_start(ctd[msl, nsl], ctt[:])
                htt = o_pool.tile([128, NCH], F32, tag="htt")
                nc.vector.tensor_add(htt[:], hdir[0][:], hdir[1][:])
                nc.sync.dma_start(htd[msl, nsl], htt[:])

            # Software-pipelined x-projections: px(mt+1) is issued before
            # dirs(mt) so the PE has ~7us of weight/x-only work to chew on
            # whenever the hidden-projection inputs (wh, h, at startup) or
            # PSUM banks lag. At kernel start px(0)+px(1) = 64 MMs cover the
            # wh0/h0 DMA window that previously left a ~9us PE gap. These
            # early start=True groups also cover all 8 PSUM banks before any
            # start=False inject group runs (defined has_written state).
            xp_store: dict = {}

            def issue_px(mt, cis=range(N_CHUNKS)):
                for ci in cis:
                    xp_store[(mt, ci)] = px_phase(mt, ci, wx_tiles[mt])

            issue_px(0)
            for mt in range(M_TILES):
                msl = slice(mt * 128, (mt + 1) * 128)
                if mt + 2 < M_TILES:
                    load_w(mt + 2)
                whm = wh_tiles.pop(mt)
                wh8m = wh8_tiles.pop(mt)
                # mt=0: both px(1) chunks up front (startup filler while the
                # first h/wh DMAs land). Later mts: only chunk 0 here; chunk
                # 1 is issued between the two dir-chunk blocks below, so the
                # 8-op xp-copy burst on ScalarE is split in half (the full
                # burst starved the PE of injects at m-tile boundaries).
                if mt + 1 < M_TILES:
                    issue_px(mt + 1, range(N_CHUNKS) if mt == 0 else [0])

                # Prefetch this m-tile's c_prev chunks up front: issued from
                # inside dir_phase they have only ~1.7us of lead and a busy
                # DMA queue stalls the f*c_prev op, which holds gate tiles
                # and ultimately the PSUM drain.
                cps = {}
                for ci in range(N_CHUNKS):
                    nsl = slice(ci * NCH, (ci + 1) * NCH)
                    for d in (0, 1):
                        cp = cp_pool.tile(
                            [128, NCH], F32, tag="cp", name=f"cp_{mt}_{ci}_{d}"
                        )
                        nc.sync.dma_start(cp[:], cd_[d][msl, nsl])
                        cps[(ci, d)] = cp

                for ci in range(N_CHUNKS):
                    xps = xp_store.pop((mt, ci))
                    c0, h0 = dir_phase(mt, ci, 0, xps, whm, wh8m, msl, cps[(ci, 0)])
                    c1, h1 = dir_phase(mt, ci, 1, xps, whm, wh8m, msl, cps[(ci, 1)])
                    combine(ci, msl, [c0, c1], [h0, h1])
                    if ci == 0 and 1 <= mt and mt + 1 < M_TILES:
                        issue_px(mt + 1, [1])

    nc.finalize()
    n_mm = sum(
        1 for i in nc.inst_map.values() if type(i).__name__ == "InstMatmult"
    )
    expected_mm = 2 * M_TILES * (4 * KX + 2 * (P8_TOT + F16_TOT)) + 24
    assert n_mm == expected_mm, f"matmul count {n_mm} != {expected_mm}"
    return nc


_CACHE: dict = {}


def _get_nc(ws0: float, ws1: float):
    key = (ws0, ws1)
    if key not in _CACHE:
        _CACHE.clear()
        _CACHE[key] = _build(ws0, ws1)
    return _CACHE[key]


F8_NP = _mld.float8_e4m3  # TRN fp8e4: IEEE e4m3, max normal +-240


def _prep_w(w: np.ndarray, kt: int, np_dt=None) -> np.ndarray:
    """(K, OUT_C)-transposed weight rows -> [m_tile, partition, k_tile, m_in_tile]."""
    wT = w
    k = wT.shape[0]
    assert k == kt * 128
    r = wT.reshape(kt, 128, M_TILES, 128)  # [ktile, p, mtile, mi]
    return np.ascontiguousarray(
        r.transpose(2, 1, 0, 3).astype(np_dt if np_dt is not None else MM_NP)
    )


def _prep_rhs(a: np.ndarray, kt: int, np_dt=None) -> np.ndarray:
    """(K, n) activation rows -> [partition, k_tile, n]."""
    k, n = a.shape
    assert k == kt * 128
    return np.ascontiguousarray(
        a.reshape(kt, 128, n).transpose(1, 0, 2).astype(
            np_dt if np_dt is not None else MM_NP
        )
    )


def run(inputs: dict, trace: bool = False, trace_kwargs: dict | None = None):
    x = np.asarray(inputs["x"], dtype=np.float32)
    ws = np.asarray(inputs["weighted_sum"], dtype=np.float32)
    ws0, ws1 = float(ws[0]), float(ws[1])
    nc = _get_nc(ws0, ws1)

    # weight hosts: [m_tile, partition, gate, k_tile, m_in_tile]
    wx_host = np.ascontiguousarray(np.stack(
        [_prep_w(np.ascontiguousarray(np.asarray(inputs[k], dtype=np.float32).T), KX)
         for k in ("w_ii", "w_if", "w_ig", "w_io")],
        axis=2,
    ))
    whT = [
        np.ascontiguousarray(np.asarray(inputs[k], dtype=np.float32).T)
        for k in ("w_hi", "w_hf", "w_hg", "w_ho")
    ]
    # fp16 part: each gate's trailing KHF_G[g] k-tiles, packed along the
    # k-tile axis -> [mt, p, F16_TOT, mi]
    wh_host = np.ascontiguousarray(np.concatenate(
        [_prep_w(whT[g][NH8[g] * 128:], KHF_G[g]) for g in range(4) if KHF_G[g]],
        axis=2,
    ))
    # fp8 part: each gate's leading NH8[g] k-tiles grouped into DoubleRow
    # pairs -> [mt, p, P8_TOT, 2, mi]
    def _pairs(g):
        r = _prep_w(whT[g][: NH8[g] * 128], NH8[g], F8_NP)  # [mt,p,kt,mi]
        s = r.shape
        return r.reshape(s[0], s[1], PAIRS[g], 2, s[3])
    wh8_host = np.ascontiguousarray(np.concatenate(
        [_pairs(g) for g in range(4) if PAIRS[g]], axis=2,
    ))
    bias_host = np.concatenate(
        [np.asarray(inputs[k], dtype=np.float32).reshape(M_TILES, 128).T
         for k in ("b_i", "b_f", "b_g", "b_o")],
        axis=1,
    )
    bias_host = np.ascontiguousarray(bias_host)

    h0 = np.asarray(inputs["h_prev_dim0"], dtype=np.float32)
    h1 = np.asarray(inputs["h_prev_dim1"], dtype=np.float32)
    c0 = np.asarray(inputs["c_prev_dim0"], dtype=np.float32)
    c1 = np.asarray(inputs["c_prev_dim1"], dtype=np.float32)

    in_maps = []
    for core in range(N_CORES):
        csl = slice(core * NS, (core + 1) * NS)
        m = {
            "c0": np.ascontiguousarray(c0[:, csl]),
            "c1": np.ascontiguousarray(c1[:, csl]),
            "wx": wx_host,
            "wh": wh_host,
            "wh8": wh8_host,
            "bias": bias_host,
        }
        xr = _prep_rhs(x[:, csl], KX)
        hr = [_prep_rhs(h0[:, csl], KH), _prep_rhs(h1[:, csl], KH)]
        h8r = [
            _prep_rhs(h0[:, csl], KH, F8_NP),
            _prep_rhs(h1[:, csl], KH, F8_NP),
        ]
        for n in range(N_CHUNKS):
            nsl = slice(n * NCH, (n + 1) * NCH)
            m[f"x{n}"] = np.ascontiguousarray(xr[:, :, nsl])
            for d in (0, 1):
                m[f"h{d}_{n}"] = np.ascontiguousarray(hr[d][:, :, nsl])
                m[f"h8_{d}_{n}"] = np.ascontiguousarray(h8r[d][:, :, nsl])
        in_maps.append(m)

    res = run_bass_kernel_spmd(
        nc,
        in_maps,
        list(range(N_CORES)),
        trace=trace,
        **(trace_kwargs or {}),
    )
    ct = np.concatenate([res.results[c]["ct"] for c in range(N_CORES)], axis=1)
    ht = np.concatenate([res.results[c]["ht"] for c in range(N_CORES)], axis=1)
    return (ct, ht), res


def kernel(**inputs) -> tuple:
    (ct, ht), _ = run(inputs)
    return ct, ht

